# revision 1
# baseline (speedup 1.0000x reference)
"""Trainium2 Bass kernel for nn_Contour_to_mask (winding-number soft
rasterization of a 128-point contour into a (1, 2, 256, 256) f32 mask).

Math: for pixel m = (mx, my) = (i/256, j/256) and edge (c_n, c_{n+1}):
  cross_n(m) = (cy*cxn - cx*cyn) + (cyn-cy)*mx + (cx-cxn)*my
  dot_n(m)   = (cx*cxn + cy*cyn) - (cx+cxn)*mx - (cy+cyn)*my + mx^2 + my^2
Both are SEPARABLE into per-edge row/column profiles:
  cross[n, i, j] = Pc[n, i] + Qc[n, j];   dot[n, i, j] = Rd[n, i] + Sd[n, j].
  angle = arccos(clip(cos, -1+eps, 1-eps)) == pi/2 - arctan(clip(r, +-R1))
  with r = dot/|cross|, R1 = cot(arccos(1-eps)).
  contribution = tanh(1e5*cross)*angle; winding = |sum_n contrib|/2pi, clip.

Engine split per 2048-pixel (8-image-row) superblock (partitions = 128 edges):
  ACT:  4 cross row-builds (Identity w/ per-partition bias), s = tanh(1e5*
        cross) -> bf16, phi = arctan(rc). Tanh+Arctan share one table set.
  GPSIMD: 4 cross row-builds (tensor_scalar add), t2 = s*phi.
  DVE:  v = reciprocal_approx_fast(cross); per-row custom fused op
        rc = clip((Sd + Rd_i)*|v|, +-R1) that BUILDS dot inline (Sd tensor +
        per-partition scalar Rd_i) and guards NaN via select(m==m)
        (cross==+-0 -> v=NaN -> rc:=R1; s=tanh(0)=0 kills it, matching ref).
  PE:   reduction over the 128 edges via sliding-window one-hot lhsT matmuls
        into two PSUM tiles: accS = sum(s) (bf16 rhs, full-rate) and
        accT = sum(t2) (fp32 rhs); finale w = min(|pi/2*accS - accT|/2pi, 1).

Sharding: 8 cores; core c handles batch c//4, image rows [(c%4)*64, +64).
"""
import sys

sys.path.insert(0, "/opt/trn_rl_repo")

import numpy as np

SIZE = 256
K_TANH = 100000.0
EPS = 1e-5
B = 2
NPTS = 128
N_CORES = 8
PIX = SIZE * SIZE              # 65536
PIX_CORE = PIX * B // N_CORES  # 16384 pixels per core
ROWS_CORE = PIX_CORE // SIZE   # 64 image rows per core
BLK = 512                      # pixels per reduction block (one PSUM bank)
NBLK = PIX_CORE // BLK         # 32
SBLK = 2048                    # pixels per elementwise superblock (8 rows)
NSBLK = PIX_CORE // SBLK       # 8

_compiled = {}
_ops = {}


def _clip_bound():
    c = np.float64(np.float32(1.0 - EPS))
    return np.float32(c / np.sqrt(1.0 - c * c))


def _register_ops():
    """Register the two custom DVE ops (idempotent)."""
    if _ops:
        return _ops
    from concourse import dve_ops
    from concourse.dve_spec import (
        Spec, Src0, Src1, C0, C1, C2, Zero, maxx, minn, eq, select, lower)
    from concourse.dve_uop import DveOpSpec

    def reg(name, spec):
        if name in dve_ops._SUB_OPCODE_FOR_NAME:
            return next(op for op in dve_ops.OPS if op.name == name)
        row = dve_ops._CUSTOM_DVE_ROW_BASE + len(dve_ops.OPS)
        sha = {ver: DveOpSpec(name=name, opcode=row,
                              uops=lower(spec, ver=ver), rd1_en=True).sha(ver)
               for ver in ("v3", "v4")}
        op = dve_ops.DveOp(name, spec, subdim=False, uops_sha=sha)
        dve_ops.OPS.append(op)
        dve_ops.CUSTOM_DVE_SPECS[name] = spec
        dve_ops._SUB_OPCODE_FOR_NAME[name] = row
        return op

    # DOT_RMUL_CLIP: out = clip((Src1 + C0)*|Src0|, -C1, C1); NaN -> C1.
    # Src0 = v (recip of cross), Src1 = Sd column profile, C0 = Rd_i scalar.
    _d = Src1 + C0
    _av = maxx(Src0, Zero - Src0)
    _m = _d * _av
    _cl = minn(maxx(_m, Zero - C1), C1)
    _body = select(eq(_m, _m), _cl, C1)

    def _ref_rmul(in0, in1, s0, s1, imm2):
        m = (in1 + s0) * np.abs(in0)
        out = np.minimum(np.maximum(m, -s1), s1)
        return np.where(np.isnan(m), s1, out).astype(np.float32)

    _ops["rmul"] = reg("DOT_RMUL_CLIP", Spec(body=_body, reference=_ref_rmul))

    # FINALE: out = min(|Src0*C0 - Src1| * C1, C2)
    _fd = Src0 * C0 - Src1
    _fa = maxx(_fd, Zero - _fd)
    _fbody = minn(_fa * C1, C2)

    def _ref_fin(in0, in1, s0, s1, imm2):
        return np.minimum(np.abs(in0 * s0 - in1) * s1, imm2).astype(np.float32)

    _ops["fin"] = reg("WINDING_FINALE", Spec(body=_fbody, reference=_ref_fin))
    return _ops


def _build(repeat=1):
    import concourse.bacc as bacc
    import concourse.tile as tile
    import concourse.mybir as mybir

    AF = mybir.ActivationFunctionType
    ALU = mybir.AluOpType
    f32 = mybir.dt.float32
    bf16 = mybir.dt.bfloat16
    ops = _register_ops()

    nc = bacc.Bacc("TRN2", target_bir_lowering=False, debug=False,
                   num_devices=N_CORES)

    pc_d = nc.dram_tensor("pc", [NPTS, ROWS_CORE], f32, kind="ExternalInput").ap()
    qc_d = nc.dram_tensor("qc", [NPTS, SIZE], f32, kind="ExternalInput").ap()
    rd_d = nc.dram_tensor("rd", [NPTS, ROWS_CORE], f32, kind="ExternalInput").ap()
    sd_d = nc.dram_tensor("sd", [NPTS, SIZE], f32, kind="ExternalInput").ap()
    redp_d = nc.dram_tensor("redp", [NPTS, 63], bf16, kind="ExternalInput").ap()
    redm_d = nc.dram_tensor("redm", [NPTS, 63], f32, kind="ExternalInput").ap()
    out_d = nc.dram_tensor("out", [NBLK, BLK], f32, kind="ExternalOutput").ap()

    R1 = float(_clip_bound())
    RPB = SBLK // SIZE  # rows per superblock = 8
    BPB = SBLK // BLK   # reduction blocks per superblock = 4

    with tile.TileContext(nc) as tc:
        with tc.tile_pool(name="cst", bufs=1) as cst, \
             tc.tile_pool(name="work", bufs=3) as work, \
             tc.tile_pool(name="pacc", bufs=1, space="PSUM") as pacc:
            pc_t = cst.tile([NPTS, ROWS_CORE], f32, name="pc_t")
            qc_t = cst.tile([NPTS, SIZE], f32, name="qc_t")
            rd_t = cst.tile([NPTS, ROWS_CORE], f32, name="rd_t")
            sd_t = cst.tile([NPTS, SIZE], f32, name="sd_t")
            redp_t = cst.tile([NPTS, 63], bf16, name="redp_t")
            redm_t = cst.tile([NPTS, 63], f32, name="redm_t")
            nc.sync.dma_start(pc_t[:], pc_d[:])
            nc.sync.dma_start(qc_t[:], qc_d[:])
            nc.sync.dma_start(rd_t[:], rd_d[:])
            nc.sync.dma_start(sd_t[:], sd_d[:])
            nc.sync.dma_start(redp_t[:], redp_d[:])
            nc.sync.dma_start(redm_t[:], redm_d[:])

            accS = pacc.tile([NBLK, BLK], f32, name="accS")
            accT = pacc.tile([NBLK, BLK], f32, name="accT")

            for rep in range(repeat):
                for u in range(NSBLK):
                    cross = work.tile([NPTS, SBLK], f32, tag="cross",
                                      name=f"cross{rep}_{u}")
                    for h in range(RPB):
                        i = u * RPB + h  # local image row
                        hs = slice(h * SIZE, (h + 1) * SIZE)
                        if h % 2 == 1:
                            nc.gpsimd.tensor_scalar(
                                cross[:, hs], qc_t[:], pc_t[:, i:i + 1], None,
                                ALU.add)
                        else:
                            nc.vector.tensor_scalar(
                                cross[:, hs], qc_t[:], pc_t[:, i:i + 1], None,
                                ALU.add)

                    s = work.tile([NPTS, SBLK], bf16, tag="s", name=f"s{rep}_{u}")
                    for g in range(2):
                        gs = slice(g * (SBLK // 2), (g + 1) * (SBLK // 2))
                        nc.scalar.activation(s[:, gs], cross[:, gs], AF.Tanh,
                                             scale=K_TANH)

                    v = work.tile([NPTS, SBLK], f32, tag="v", name=f"v{rep}_{u}")
                    nc.vector.reciprocal_approx_fast(v[:], cross[:])

                    rc = work.tile([NPTS, SBLK], f32, tag="rc", name=f"rc{rep}_{u}")
                    for h in range(RPB):
                        i = u * RPB + h
                        hs = slice(h * SIZE, (h + 1) * SIZE)
                        nc.vector._custom_dve(
                            ops["rmul"], out=rc[:, hs], in0=v[:, hs],
                            in1=sd_t[:], s0=rd_t[:, i:i + 1], s1=R1)

                    phi = work.tile([NPTS, SBLK], f32, tag="phi",
                                    name=f"phi{rep}_{u}")
                    for g in range(2):
                        gs = slice(g * (SBLK // 2), (g + 1) * (SBLK // 2))
                        nc.scalar.activation(phi[:, gs], rc[:, gs], AF.Arctan)

                    t2 = work.tile([NPTS, SBLK], f32, tag="t2", name=f"t2{rep}_{u}")
                    for g in range(4):
                        gs = slice(g * (SBLK // 4), (g + 1) * (SBLK // 4))
                        nc.gpsimd.tensor_tensor(t2[:, gs], s[:, gs], phi[:, gs],
                                                ALU.mult)

                    for h in range(BPB):
                        j = BPB * u + h
                        hs = slice(h * BLK, (h + 1) * BLK)
                        lp = redp_t[:, 31 - j:63 - j]
                        lm = redm_t[:, 31 - j:63 - j]
                        nc.tensor.matmul(accS[:], lp, s[:, hs],
                                         start=(j == 0), stop=False)
                        nc.tensor.matmul(accT[:], lm, t2[:, hs],
                                         start=(j == 0),
                                         stop=(j == NBLK - 1 and
                                               rep == repeat - 1))

            tcopy = work.tile([NBLK, BLK], f32, tag="tcopy", name="tcopy")
            nc.vector.tensor_copy(tcopy[:], accT[:])
            w = work.tile([NBLK, BLK], f32, tag="w", name="w")
            nc.vector._custom_dve(
                ops["fin"], out=w[:], in0=accS[:], in1=tcopy[:],
                s0=float(np.float32(np.pi / 2)),
                s1=float(np.float32(1.0 / (2.0 * np.pi))), imm2=1.0)
            nc.sync.dma_start(out_d[:], w[:])

    nc.compile()
    return nc


def _host_inputs(contour: np.ndarray):
    """Per-core in_maps from the full (B, NPTS, 2) contour."""
    mx = (np.arange(SIZE) / SIZE).astype(np.float64)   # i profile
    my = (np.arange(SIZE) / SIZE).astype(np.float64)   # j profile

    prof = []
    for b in range(B):
        cx = contour[b, :, 0].astype(np.float64)
        cy = contour[b, :, 1].astype(np.float64)
        cxn = np.roll(cx, -1)
        cyn = np.roll(cy, -1)
        A = cy * cxn - cx * cyn
        Bc = cyn - cy
        Cc = cx - cxn
        Dd = cx * cxn + cy * cyn
        Ed = -(cx + cxn)
        Fd = -(cy + cyn)
        Pc = (A[:, None] + Bc[:, None] * mx[None, :]).astype(np.float32)
        Qc = (Cc[:, None] * my[None, :]).astype(np.float32)
        Rd = (Dd[:, None] + Ed[:, None] * mx[None, :] + mx[None, :] ** 2
              ).astype(np.float32)
        Sd = (Fd[:, None] * my[None, :] + my[None, :] ** 2).astype(np.float32)
        prof.append((Pc, Qc, Rd, Sd))

    import ml_dtypes
    redp = np.zeros((NPTS, 63), dtype=ml_dtypes.bfloat16)
    redp[:, 31] = 1.0
    redm = np.zeros((NPTS, 63), dtype=np.float32)
    redm[:, 31] = 1.0

    in_maps = []
    for c in range(N_CORES):
        b = c // (N_CORES // B)
        r0 = (c % (N_CORES // B)) * ROWS_CORE
        Pc, Qc, Rd, Sd = prof[b]
        in_maps.append({
            "pc": np.ascontiguousarray(Pc[:, r0:r0 + ROWS_CORE]),
            "qc": Qc,
            "rd": np.ascontiguousarray(Rd[:, r0:r0 + ROWS_CORE]),
            "sd": Sd,
            "redp": redp,
            "redm": redm,
        })
    return in_maps


def kernel(contour: np.ndarray) -> np.ndarray:
    from concourse import bass_utils

    contour = np.asarray(contour, dtype=np.float32)
    if "nc" not in _compiled:
        _compiled["nc"] = _build()
    in_maps = _host_inputs(contour)
    res = bass_utils.run_bass_kernel_spmd(
        _compiled["nc"], in_maps, core_ids=list(range(N_CORES))).results

    mask = np.zeros((1, B, SIZE, SIZE), dtype=np.float32)
    for c in range(N_CORES):
        b = c // (N_CORES // B)
        r0 = (c % (N_CORES // B)) * ROWS_CORE
        mask[0, b, r0:r0 + ROWS_CORE, :] = (
            res[c]["out"].reshape(ROWS_CORE, SIZE))
    return mask



# revision 6
# speedup vs baseline: 1.5235x; 1.5235x over previous
"""Trainium2 Bass kernel for nn_Contour_to_mask (winding-number soft
rasterization of a 128-point contour into a (1, 2, 256, 256) f32 mask).

Math: for pixel m = (mx, my) = (i/256, j/256) and edge (c_n, c_{n+1}):
  cross_n(m) = Pc[n, i] + Qc[n, j]   (separable row/col profiles)
  dot_n(m)   = Rd[n, i] + Sd[n, j]
  reference contribution = tanh(K*cross) * (pi/2 - arctan(dot/|cross|)).
This kernel uses the sign-folded identity
  sgn(c)*arctan(d/|c|) = arctan(d/c):
  contribution ~= s*pi/2 - arctan(d/c),  s = clip((K/a)*c, +-1)
so no per-element s*angle product is needed; the only transcendental is
one arctan per (pixel, edge).

Engine split per 2048-pixel (8-image-row) superblock (partitions = 128
edges):
  ACT:  3 cross row-builds (Identity w/ per-partition bias), phi =
        arctan(rc) -> float32r (full-rate PE dtype).
  DVE:  1 cross row-build, u = clip(cross, +-a/K) -> bf16 (one
        2-op tensor_scalar, 2x perf mode), 8x FUSED_RECIP_DOT custom op:
        rc = (Sd + Rd_i) * recip1nr(cross)  (bitwise-NOT seed + one
        Newton step + dot-build + multiply fused in one 7-stage op).
  Pool: 4 cross row-builds, tail half of u.
  PE:   per 512-pixel block, two one-shot matmuls with ones lhsT:
        accS[j] = sum_n u, accT[j] = sum_n phi (bf16 and f32r, both
        full rate).  Finale per 16-row half: w = min(|s0*accS - accT|
        * 1/2pi, 1) via custom DVE op.

Host side: profiles Pc/Qc/Rd/Sd in f64 -> f32; exact-zero crossings of
Pc[n,i] + Qc[n,j] (which would NaN the reciprocal seed) are killed by
bumping Pc one ulp (error ~1e-3 of one tanh at that pixel).

Sharding: 8 cores; core c handles batch c//4, image rows [(c%4)*64, +64).
"""
import sys

sys.path.insert(0, "/opt/trn_rl_repo")

import numpy as np

SIZE = 256
K_TANH = 100000.0
A_SLOPE = 0.7            # clip slope tuning: s = clip((K/a) c, +-1)
EPS = 1e-5
B = 2
NPTS = 128
N_CORES = 8
PIX = SIZE * SIZE              # 65536
PIX_CORE = PIX * B // N_CORES  # 16384 pixels per core
ROWS_CORE = PIX_CORE // SIZE   # 64 image rows per core
BLK = 512                      # pixels per reduction block (one PSUM row)
NBLK = PIX_CORE // BLK         # 32
SBLK = 2048                    # pixels per elementwise superblock (8 rows)
NSBLK = PIX_CORE // SBLK       # 8
RPB = SBLK // SIZE             # rows per superblock = 8
BPB = SBLK // BLK              # reduction blocks per superblock = 4

# 1-NR reciprocal seed constants (bitwise-NOT exponent-flip trick).
RECIP_C0 = -0.23549792
RECIP_C1 = 2.0017324

_compiled = {}
_ops = {}


def _register_ops():
    """Register the custom DVE ops (idempotent)."""
    if _ops:
        return _ops
    from concourse import dve_ops
    from concourse.dve_spec import (
        Spec, Src0, Src1, C0, C1, C2, Zero, maxx, minn, lower)
    from concourse.dve_uop import DveOpSpec
    from concourse.dve_ops import _not_x

    def reg(name, spec):
        if name in dve_ops._SUB_OPCODE_FOR_NAME:
            return next(op for op in dve_ops.OPS if op.name == name)
        row = dve_ops._CUSTOM_DVE_ROW_BASE + len(dve_ops.OPS)
        sha = {ver: DveOpSpec(name=name, opcode=row,
                              uops=lower(spec, ver=ver), rd1_en=True).sha(ver)
               for ver in ("v3", "v4")}
        op = dve_ops.DveOp(name, spec, subdim=False, uops_sha=sha)
        dve_ops.OPS.append(op)
        dve_ops.CUSTOM_DVE_SPECS[name] = spec
        dve_ops._SUB_OPCODE_FOR_NAME[name] = row
        return op

    # FUSED_RECIP_DOT: out = (Src1 + C0) * y1 where y1 is a 1-Newton-step
    # reciprocal of Src0 (C1 = seed scale, C2 = NR constant).
    _y0 = _not_x * C1
    _y1 = _y0 * (C2 - Src0 * _y0)

    def _ref_frd(in0, in1, s0, s1, imm2):
        not_x = (~in0.view(np.int32)).view(np.float32)
        y0 = not_x * np.float32(s1)
        y1 = (y0 * (np.float32(imm2) - in0 * y0)).astype(np.float32)
        return ((in1 + s0) * y1).astype(np.float32)

    _ops["frd"] = reg("FUSED_RECIP_DOT",
                      Spec(body=(Src1 + C0) * _y1, reference=_ref_frd))

    # FINALE: out = min(|Src0*C0 - Src1| * C1, C2)
    _fd = Src0 * C0 - Src1
    _fa = maxx(_fd, Zero - _fd)
    _fbody = minn(_fa * C1, C2)

    def _ref_fin(in0, in1, s0, s1, imm2):
        return np.minimum(np.abs(in0 * s0 - in1) * s1, imm2).astype(np.float32)

    _ops["fin"] = reg("WINDING_FINALE", Spec(body=_fbody, reference=_ref_fin))
    return _ops


def _build():
    import concourse.bacc as bacc
    import concourse.tile as tile
    import concourse.mybir as mybir

    AF = mybir.ActivationFunctionType
    ALU = mybir.AluOpType
    f32 = mybir.dt.float32
    f32r = mybir.dt.float32r
    bf16 = mybir.dt.bfloat16
    ops = _register_ops()

    nc = bacc.Bacc("TRN2", target_bir_lowering=False, debug=False,
                   num_devices=N_CORES)

    pc_d = nc.dram_tensor("pc", [NPTS, ROWS_CORE], f32, kind="ExternalInput").ap()
    qc_d = nc.dram_tensor("qc", [NPTS, SIZE], f32, kind="ExternalInput").ap()
    rd_d = nc.dram_tensor("rd", [NPTS, ROWS_CORE], f32, kind="ExternalInput").ap()
    sd_d = nc.dram_tensor("sd", [NPTS, SIZE], f32, kind="ExternalInput").ap()
    onb_d = nc.dram_tensor("onb", [NPTS, 31], bf16, kind="ExternalInput").ap()
    onr_d = nc.dram_tensor("onr", [NPTS, 31], f32r, kind="ExternalInput").ap()
    out_d = nc.dram_tensor("out", [NBLK, BLK], f32, kind="ExternalOutput").ap()

    UB = float(np.float32(A_SLOPE / K_TANH))   # clip bound on cross
    FIN_S0 = float(np.float32((np.pi / 2) * (K_TANH / A_SLOPE)))
    FIN_S1 = float(np.float32(1.0 / (2.0 * np.pi)))
    UCOLS_DVE = 1024                           # u columns computed on DVE

    with tile.TileContext(nc) as tc:
        with tc.tile_pool(name="cst", bufs=1) as cst, \
             tc.tile_pool(name="work", bufs=3) as work, \
             tc.tile_pool(name="fin", bufs=2) as finp, \
             tc.tile_pool(name="pacc", bufs=1, space="PSUM") as pacc:
            pc_t = cst.tile([NPTS, ROWS_CORE], f32, name="pc_t")
            qc_t = cst.tile([NPTS, SIZE], f32, name="qc_t")
            rd_t = cst.tile([NPTS, ROWS_CORE], f32, name="rd_t")
            sd_t = cst.tile([NPTS, SIZE], f32, name="sd_t")
            onb_t = cst.tile([NPTS, 31], bf16, name="onb_t")
            onr_t = cst.tile([NPTS, 31], f32r, name="onr_t")
            nc.sync.dma_start(pc_t[:], pc_d[:])
            nc.sync.dma_start(qc_t[:], qc_d[:])
            nc.sync.dma_start(rd_t[:], rd_d[:])
            nc.sync.dma_start(sd_t[:], sd_d[:])
            nc.sync.dma_start(onb_t[:], onb_d[:])
            nc.sync.dma_start(onr_t[:], onr_d[:])

            # Two psum pairs: blocks 0-15 accumulate in pair a (read out
            # while superblocks 4-7 run), blocks 16-31 in pair b.
            HB = NBLK // 2  # 16 blocks per half
            accS = [pacc.tile([HB, BLK], f32, name=f"accS{x}")
                    for x in range(2)]
            accT = [pacc.tile([HB, BLK], f32, name=f"accT{x}")
                    for x in range(2)]

            for u in range(NSBLK):
                cross = work.tile([NPTS, SBLK], f32, tag="cross",
                                  name=f"cross{u}")
                for h in range(RPB):
                    i = u * RPB + h  # local image row
                    hs = slice(h * SIZE, (h + 1) * SIZE)
                    if h < 3:
                        nc.scalar.activation(cross[:, hs], qc_t[:],
                                             AF.Identity,
                                             bias=pc_t[:, i:i + 1])
                    elif h == 3:
                        nc.vector.tensor_scalar(
                            cross[:, hs], qc_t[:], pc_t[:, i:i + 1], None,
                            ALU.add)
                    else:
                        nc.gpsimd.tensor_scalar(
                            cross[:, hs], qc_t[:], pc_t[:, i:i + 1], None,
                            ALU.add)

                ut = work.tile([NPTS, SBLK], bf16, tag="ut", name=f"ut{u}")
                nc.vector.tensor_scalar(ut[:, :UCOLS_DVE],
                                        cross[:, :UCOLS_DVE],
                                        -UB, UB, ALU.max, ALU.min)
                nc.gpsimd.tensor_scalar(ut[:, UCOLS_DVE:],
                                        cross[:, UCOLS_DVE:],
                                        -UB, UB, ALU.max, ALU.min)

                rc = work.tile([NPTS, SBLK], f32, tag="rc", name=f"rc{u}")
                for h in range(RPB):
                    i = u * RPB + h
                    hs = slice(h * SIZE, (h + 1) * SIZE)
                    nc.vector._custom_dve(
                        ops["frd"], out=rc[:, hs], in0=cross[:, hs],
                        in1=sd_t[:], s0=rd_t[:, i:i + 1],
                        s1=RECIP_C0, imm2=RECIP_C1)

                phi = work.tile([NPTS, SBLK], f32r, tag="phi", name=f"phi{u}")
                for g in range(2):
                    gs = slice(g * (SBLK // 2), (g + 1) * (SBLK // 2))
                    nc.scalar.activation(phi[:, gs], rc[:, gs], AF.Arctan)

                x = u // 4           # psum pair for this half
                for h in range(BPB):
                    j = BPB * u + h
                    jh = j % HB      # block row within the half
                    hs = slice(h * BLK, (h + 1) * BLK)
                    lb = onb_t[:, 15 - jh:31 - jh]
                    lr = onr_t[:, 15 - jh:31 - jh]
                    nc.tensor.matmul(accS[x][:], lb, ut[:, hs],
                                     start=(jh == 0), stop=(jh == HB - 1))
                    nc.tensor.matmul(accT[x][:], lr, phi[:, hs],
                                     start=(jh == 0), stop=(jh == HB - 1))

                if u % 4 == 3:
                    half = slice(x * HB, x * HB + HB)
                    tcp = finp.tile([HB, BLK], f32, tag="tcp",
                                    name=f"tcp{x}")
                    nc.vector.tensor_copy(tcp[:], accT[x][:])
                    w = finp.tile([HB, BLK], f32, tag="w", name=f"w{x}")
                    nc.vector._custom_dve(
                        ops["fin"], out=w[:], in0=accS[x][:], in1=tcp[:],
                        s0=FIN_S0, s1=FIN_S1, imm2=1.0)
                    nc.sync.dma_start(out_d[half, :], w[:])

    nc.compile()
    return nc


def _host_inputs(contour: np.ndarray):
    """Per-core in_maps from the full (B, NPTS, 2) contour."""
    import ml_dtypes

    mx = (np.arange(SIZE) / SIZE).astype(np.float64)   # i profile
    my = (np.arange(SIZE) / SIZE).astype(np.float64)   # j profile

    prof = []
    for b in range(B):
        cx = contour[b, :, 0].astype(np.float64)
        cy = contour[b, :, 1].astype(np.float64)
        cxn = np.roll(cx, -1)
        cyn = np.roll(cy, -1)
        A = cy * cxn - cx * cyn
        Bc = cyn - cy
        Cc = cx - cxn
        Dd = cx * cxn + cy * cyn
        Ed = -(cx + cxn)
        Fd = -(cy + cyn)
        Pc = (A[:, None] + Bc[:, None] * mx[None, :]).astype(np.float32)
        Qc = (Cc[:, None] * my[None, :]).astype(np.float32)
        Rd = (Dd[:, None] + Ed[:, None] * mx[None, :] + mx[None, :] ** 2
              ).astype(np.float32)
        Sd = (Fd[:, None] * my[None, :] + my[None, :] ** 2).astype(np.float32)
        # Kill exact zeros of Pc[n,i] + Qc[n,j]: they would NaN the
        # bitwise-NOT reciprocal seed on-device.
        for _ in range(4):
            c = Pc[:, :, None] + Qc[:, None, :]
            nz, iz, _jz = np.nonzero(c == 0.0)
            if len(nz) == 0:
                break
            for n, i in set(zip(nz.tolist(), iz.tolist())):
                Pc[n, i] = np.nextafter(Pc[n, i], np.float32(np.inf),
                                        dtype=np.float32)
        prof.append((Pc, Qc, Rd, Sd))

    onb = np.zeros((NPTS, 31), dtype=ml_dtypes.bfloat16)
    onb[:, 15] = 1.0
    onr = np.zeros((NPTS, 31), dtype=np.float32)
    onr[:, 15] = 1.0

    in_maps = []
    for c in range(N_CORES):
        b = c // (N_CORES // B)
        r0 = (c % (N_CORES // B)) * ROWS_CORE
        Pc, Qc, Rd, Sd = prof[b]
        in_maps.append({
            "pc": np.ascontiguousarray(Pc[:, r0:r0 + ROWS_CORE]),
            "qc": Qc,
            "rd": np.ascontiguousarray(Rd[:, r0:r0 + ROWS_CORE]),
            "sd": Sd,
            "onb": onb,
            "onr": onr,
        })
    return in_maps


def kernel(contour: np.ndarray) -> np.ndarray:
    from concourse import bass_utils

    contour = np.asarray(contour, dtype=np.float32)
    if "nc" not in _compiled:
        _compiled["nc"] = _build()
    in_maps = _host_inputs(contour)
    res = bass_utils.run_bass_kernel_spmd(
        _compiled["nc"], in_maps, core_ids=list(range(N_CORES))).results

    mask = np.zeros((1, B, SIZE, SIZE), dtype=np.float32)
    for c in range(N_CORES):
        b = c // (N_CORES // B)
        r0 = (c % (N_CORES // B)) * ROWS_CORE
        mask[0, b, r0:r0 + ROWS_CORE, :] = (
            res[c]["out"].reshape(ROWS_CORE, SIZE))
    return mask


# revision 7
# speedup vs baseline: 1.5546x; 1.0204x over previous
"""Trainium2 Bass kernel for nn_Contour_to_mask (winding-number soft
rasterization of a 128-point contour into a (1, 2, 256, 256) f32 mask).

Math: for pixel m = (mx, my) = (i/256, j/256) and edge (c_n, c_{n+1}):
  cross_n(m) = Pc[n, i] + Qc[n, j]   (separable row/col profiles)
  dot_n(m)   = Rd[n, i] + Sd[n, j]
  reference contribution = tanh(K*cross) * (pi/2 - arctan(dot/|cross|)).
This kernel uses the sign-folded identity
  sgn(c)*arctan(d/|c|) = arctan(d/c):
  contribution ~= s*pi/2 - arctan(d/c),  s = clip((K/a)*c, +-1)
so no per-element s*angle product is needed; the only transcendental is
one arctan per (pixel, edge).

Engine split per 2048-pixel (8-image-row) superblock (partitions = 128
edges):
  ACT:  3 cross row-builds (Identity w/ per-partition bias), phi =
        arctan(rc) -> float32r (full-rate PE dtype).
  DVE:  1 cross row-build, u = clip(cross, +-a/K) -> bf16 (one
        2-op tensor_scalar, 2x perf mode), 8x FUSED_RECIP_DOT custom op:
        rc = (Sd + Rd_i) * recip1nr(cross)  (bitwise-NOT seed + one
        Newton step + dot-build + multiply fused in one 7-stage op).
  Pool: 4 cross row-builds, tail half of u.
  PE:   per 512-pixel block, two one-shot matmuls with ones lhsT:
        accS[j] = sum_n u, accT[j] = sum_n phi (bf16 and f32r, both
        full rate).  Finale per 16-row half: w = min(|s0*accS - accT|
        * 1/2pi, 1) via custom DVE op.

Host side: profiles Pc/Qc/Rd/Sd in f64 -> f32; exact-zero crossings of
Pc[n,i] + Qc[n,j] (which would NaN the reciprocal seed) are killed by
bumping Pc one ulp (error ~1e-3 of one tanh at that pixel).

Sharding: 8 cores; core c handles batch c//4, image rows [(c%4)*64, +64).
"""
import sys

sys.path.insert(0, "/opt/trn_rl_repo")

import numpy as np

SIZE = 256
K_TANH = 100000.0
A_SLOPE = 0.7            # clip slope tuning: s = clip((K/a) c, +-1)
EPS = 1e-5
B = 2
NPTS = 128
N_CORES = 8
PIX = SIZE * SIZE              # 65536
PIX_CORE = PIX * B // N_CORES  # 16384 pixels per core
ROWS_CORE = PIX_CORE // SIZE   # 64 image rows per core
BLK = 512                      # pixels per reduction block (one PSUM row)
NBLK = PIX_CORE // BLK         # 32
SBLK = 2048                    # pixels per elementwise superblock (8 rows)
NSBLK = PIX_CORE // SBLK       # 8
RPB = SBLK // SIZE             # rows per superblock = 8
BPB = SBLK // BLK              # reduction blocks per superblock = 4

# 1-NR reciprocal seed constants (bitwise-NOT exponent-flip trick).
RECIP_C0 = -0.23549792
RECIP_C1 = 2.0017324

_compiled = {}
_ops = {}


def _register_ops():
    """Register the custom DVE ops (idempotent)."""
    if _ops:
        return _ops
    from concourse import dve_ops
    from concourse.dve_spec import (
        Spec, Src0, Src1, C0, C1, C2, Zero, maxx, minn, lower)
    from concourse.dve_uop import DveOpSpec
    from concourse.dve_ops import _not_x

    def reg(name, spec):
        if name in dve_ops._SUB_OPCODE_FOR_NAME:
            return next(op for op in dve_ops.OPS if op.name == name)
        row = dve_ops._CUSTOM_DVE_ROW_BASE + len(dve_ops.OPS)
        sha = {ver: DveOpSpec(name=name, opcode=row,
                              uops=lower(spec, ver=ver), rd1_en=True).sha(ver)
               for ver in ("v3", "v4")}
        op = dve_ops.DveOp(name, spec, subdim=False, uops_sha=sha)
        dve_ops.OPS.append(op)
        dve_ops.CUSTOM_DVE_SPECS[name] = spec
        dve_ops._SUB_OPCODE_FOR_NAME[name] = row
        return op

    # FUSED_RECIP_DOT: out = (Src1 + C0) * y1 where y1 is a 1-Newton-step
    # reciprocal of Src0 (C1 = seed scale, C2 = NR constant).
    _y0 = _not_x * C1
    _y1 = _y0 * (C2 - Src0 * _y0)

    def _ref_frd(in0, in1, s0, s1, imm2):
        not_x = (~in0.view(np.int32)).view(np.float32)
        y0 = not_x * np.float32(s1)
        y1 = (y0 * (np.float32(imm2) - in0 * y0)).astype(np.float32)
        return ((in1 + s0) * y1).astype(np.float32)

    _ops["frd"] = reg("FUSED_RECIP_DOT",
                      Spec(body=(Src1 + C0) * _y1, reference=_ref_frd))

    # FINALE: out = min(|Src0*C0 - Src1| * C1, C2)
    _fd = Src0 * C0 - Src1
    _fa = maxx(_fd, Zero - _fd)
    _fbody = minn(_fa * C1, C2)

    def _ref_fin(in0, in1, s0, s1, imm2):
        return np.minimum(np.abs(in0 * s0 - in1) * s1, imm2).astype(np.float32)

    _ops["fin"] = reg("WINDING_FINALE", Spec(body=_fbody, reference=_ref_fin))
    return _ops


def _build():
    import concourse.bacc as bacc
    import concourse.tile as tile
    import concourse.mybir as mybir

    AF = mybir.ActivationFunctionType
    ALU = mybir.AluOpType
    f32 = mybir.dt.float32
    f32r = mybir.dt.float32r
    bf16 = mybir.dt.bfloat16
    ops = _register_ops()

    nc = bacc.Bacc("TRN2", target_bir_lowering=False, debug=False,
                   num_devices=N_CORES)

    PKW = 2 * ROWS_CORE + 2 * SIZE  # pc | qc | rd | sd packed = 640
    pk_d = nc.dram_tensor("pk", [NPTS, PKW], f32, kind="ExternalInput").ap()
    onb_d = nc.dram_tensor("onb", [NPTS, 31], bf16, kind="ExternalInput").ap()
    onr_d = nc.dram_tensor("onr", [NPTS, 31], f32r, kind="ExternalInput").ap()
    out_d = nc.dram_tensor("out", [NBLK, BLK], f32, kind="ExternalOutput").ap()

    UB = float(np.float32(A_SLOPE / K_TANH))   # clip bound on cross
    FIN_S0 = float(np.float32((np.pi / 2) * (K_TANH / A_SLOPE)))
    FIN_S1 = float(np.float32(1.0 / (2.0 * np.pi)))
    UCOLS_DVE = 1408                           # u columns computed on DVE

    with tile.TileContext(nc) as tc:
        with tc.tile_pool(name="cst", bufs=1) as cst, \
             tc.tile_pool(name="work", bufs=3) as work, \
             tc.tile_pool(name="fin", bufs=2) as finp, \
             tc.tile_pool(name="pacc", bufs=1, space="PSUM") as pacc:
            pk_t = cst.tile([NPTS, PKW], f32, name="pk_t")
            onb_t = cst.tile([NPTS, 31], bf16, name="onb_t")
            onr_t = cst.tile([NPTS, 31], f32r, name="onr_t")
            dum_t = cst.tile([NPTS, 1], f32, name="dum_t")
            dmo_t = cst.tile([NPTS, 2], f32, name="dmo_t")
            prb_t = cst.tile([NPTS, 64], bf16, name="prb_t")

            # Prime the ACT tables (Arctan + Identity) and the PE p-state
            # while the input DMAs are in flight.
            nc.vector.memset(dum_t[:], 1.0)
            nc.vector.memset(prb_t[:], 0.0)
            nc.scalar.activation(dmo_t[:, 0:1], dum_t[:], AF.Arctan)
            nc.scalar.activation(dmo_t[:, 1:2], dum_t[:], AF.Identity,
                                 bias=dum_t[:])
            prps = pacc.tile([64, 64], f32, name="prps")
            nc.tensor.matmul(prps[:], prb_t[:], prb_t[:, :64],
                             start=True, stop=True)

            nc.sync.dma_start(pk_t[:], pk_d[:])
            nc.sync.dma_start(onb_t[:], onb_d[:])
            nc.sync.dma_start(onr_t[:], onr_d[:])
            pc_t = pk_t[:, 0:ROWS_CORE]
            qc_t = pk_t[:, ROWS_CORE:ROWS_CORE + SIZE]
            rd_t = pk_t[:, ROWS_CORE + SIZE:2 * ROWS_CORE + SIZE]
            sd_t = pk_t[:, 2 * ROWS_CORE + SIZE:PKW]

            # Two psum pairs: blocks 0-15 accumulate in pair a (read out
            # while superblocks 4-7 run), blocks 16-31 in pair b.
            HB = NBLK // 2  # 16 blocks per half
            accS = [pacc.tile([HB, BLK], f32, name=f"accS{x}")
                    for x in range(2)]
            accT = [pacc.tile([HB, BLK], f32, name=f"accT{x}")
                    for x in range(2)]

            for u in range(NSBLK):
                cross = work.tile([NPTS, SBLK], f32, tag="cross",
                                  name=f"cross{u}")
                for h in range(RPB):
                    i = u * RPB + h  # local image row
                    hs = slice(h * SIZE, (h + 1) * SIZE)
                    if h < 3:
                        nc.scalar.activation(cross[:, hs], qc_t,
                                             AF.Identity,
                                             bias=pc_t[:, i:i + 1])
                    else:
                        nc.gpsimd.tensor_scalar(
                            cross[:, hs], qc_t, pc_t[:, i:i + 1], None,
                            ALU.add)

                ut = work.tile([NPTS, SBLK], bf16, tag="ut", name=f"ut{u}")
                nc.vector.tensor_scalar(ut[:, :UCOLS_DVE],
                                        cross[:, :UCOLS_DVE],
                                        -UB, UB, ALU.max, ALU.min)
                nc.gpsimd.tensor_scalar(ut[:, UCOLS_DVE:],
                                        cross[:, UCOLS_DVE:],
                                        -UB, UB, ALU.max, ALU.min)

                rc = work.tile([NPTS, SBLK], f32, tag="rc", name=f"rc{u}")
                for h in range(RPB):
                    i = u * RPB + h
                    hs = slice(h * SIZE, (h + 1) * SIZE)
                    nc.vector._custom_dve(
                        ops["frd"], out=rc[:, hs], in0=cross[:, hs],
                        in1=sd_t, s0=rd_t[:, i:i + 1],
                        s1=RECIP_C0, imm2=RECIP_C1)

                phi = work.tile([NPTS, SBLK], f32r, tag="phi", name=f"phi{u}")
                for g in range(2):
                    gs = slice(g * (SBLK // 2), (g + 1) * (SBLK // 2))
                    nc.scalar.activation(phi[:, gs], rc[:, gs], AF.Arctan)

                x = u // 4           # psum pair for this half
                for h in range(BPB):
                    j = BPB * u + h
                    jh = j % HB      # block row within the half
                    hs = slice(h * BLK, (h + 1) * BLK)
                    lb = onb_t[:, 15 - jh:31 - jh]
                    lr = onr_t[:, 15 - jh:31 - jh]
                    nc.tensor.matmul(accS[x][:], lb, ut[:, hs],
                                     start=(jh == 0), stop=(jh == HB - 1))
                    nc.tensor.matmul(accT[x][:], lr, phi[:, hs],
                                     start=(jh == 0), stop=(jh == HB - 1))

                if u % 4 == 3:
                    half = slice(x * HB, x * HB + HB)
                    tcp = finp.tile([HB, BLK], f32, tag="tcp",
                                    name=f"tcp{x}")
                    nc.vector.tensor_copy(tcp[:], accT[x][:])
                    w = finp.tile([HB, BLK], f32, tag="w", name=f"w{x}")
                    nc.vector._custom_dve(
                        ops["fin"], out=w[:], in0=accS[x][:], in1=tcp[:],
                        s0=FIN_S0, s1=FIN_S1, imm2=1.0)
                    nc.sync.dma_start(out_d[half, :], w[:])

    nc.compile()
    return nc


def _host_inputs(contour: np.ndarray):
    """Per-core in_maps from the full (B, NPTS, 2) contour."""
    import ml_dtypes

    mx = (np.arange(SIZE) / SIZE).astype(np.float64)   # i profile
    my = (np.arange(SIZE) / SIZE).astype(np.float64)   # j profile

    prof = []
    for b in range(B):
        cx = contour[b, :, 0].astype(np.float64)
        cy = contour[b, :, 1].astype(np.float64)
        cxn = np.roll(cx, -1)
        cyn = np.roll(cy, -1)
        A = cy * cxn - cx * cyn
        Bc = cyn - cy
        Cc = cx - cxn
        Dd = cx * cxn + cy * cyn
        Ed = -(cx + cxn)
        Fd = -(cy + cyn)
        Pc = (A[:, None] + Bc[:, None] * mx[None, :]).astype(np.float32)
        Qc = (Cc[:, None] * my[None, :]).astype(np.float32)
        Rd = (Dd[:, None] + Ed[:, None] * mx[None, :] + mx[None, :] ** 2
              ).astype(np.float32)
        Sd = (Fd[:, None] * my[None, :] + my[None, :] ** 2).astype(np.float32)
        # Kill exact zeros of Pc[n,i] + Qc[n,j]: they would NaN the
        # bitwise-NOT reciprocal seed on-device.
        for _ in range(4):
            c = Pc[:, :, None] + Qc[:, None, :]
            nz, iz, _jz = np.nonzero(c == 0.0)
            if len(nz) == 0:
                break
            for n, i in set(zip(nz.tolist(), iz.tolist())):
                Pc[n, i] = np.nextafter(Pc[n, i], np.float32(np.inf),
                                        dtype=np.float32)
        prof.append((Pc, Qc, Rd, Sd))

    onb = np.zeros((NPTS, 31), dtype=ml_dtypes.bfloat16)
    onb[:, 15] = 1.0
    onr = np.zeros((NPTS, 31), dtype=np.float32)
    onr[:, 15] = 1.0

    in_maps = []
    for c in range(N_CORES):
        b = c // (N_CORES // B)
        r0 = (c % (N_CORES // B)) * ROWS_CORE
        Pc, Qc, Rd, Sd = prof[b]
        pk = np.concatenate([Pc[:, r0:r0 + ROWS_CORE], Qc,
                             Rd[:, r0:r0 + ROWS_CORE], Sd],
                            axis=1).astype(np.float32)
        in_maps.append({
            "pk": np.ascontiguousarray(pk),
            "onb": onb,
            "onr": onr,
        })
    return in_maps


def kernel(contour: np.ndarray) -> np.ndarray:
    from concourse import bass_utils

    contour = np.asarray(contour, dtype=np.float32)
    if "nc" not in _compiled:
        _compiled["nc"] = _build()
    in_maps = _host_inputs(contour)
    res = bass_utils.run_bass_kernel_spmd(
        _compiled["nc"], in_maps, core_ids=list(range(N_CORES))).results

    mask = np.zeros((1, B, SIZE, SIZE), dtype=np.float32)
    for c in range(N_CORES):
        b = c // (N_CORES // B)
        r0 = (c % (N_CORES // B)) * ROWS_CORE
        mask[0, b, r0:r0 + ROWS_CORE, :] = (
            res[c]["out"].reshape(ROWS_CORE, SIZE))
    return mask
